# revision 4
# baseline (speedup 1.0000x reference)
"""Chamfer loss kernel for Trainium2 (8 NeuronCores, SPMD).

Problem: trgt [8,4096,3], pred [8,4096,3] fp32 ->
  (accuracy, complete, chamfer) scalars, where per batch b:
    d2[n,m] = ||t_n - p_m||^2
    complete_b = mean_n sqrt(min_m d2)   (target -> pred)
    accuracy_b = mean_m sqrt(min_n d2)   (pred -> target)
  and the outputs are means over b, chamfer = 0.5*(acc+comp).

Strategy (one batch per core, data-parallel over b):
  * Host prep: NEGATED distances -d2 = 2 t.p - t2 - p2 as an augmented
    K=13 bf16 matmul (hi/lo bf16 split keeps ~fp32 precision; PSUM
    accumulates fp32).  All reductions become MAX (the only cross-lane
    ALU the HW supports everywhere); signs are restored in the tail.
  * PE: 4x row-packed matmuls (K=13 in a 32-row group) produce
    [128n x 512m] fp32 PSUM tiles, two 2048-wide quads per n-chunk.
  * ACT drains each PSUM quad once: fp32 -> bf16 copy into a per-chunk
    [128, 4096] sq tile (ACT is the only engine that can convert
    without stealing DVE throughput; GPSIMD can't touch PSUM and its
    elementwise/reduce firmware is absent or ~100x too slow).
  * DVE per chunk (all bf16 2x mode): one colacc TT max accumulate
    [128,4096], then a max tree t1/t2/t3 + 512-wide reduce for the
    row (min over m) direction.  This is the measured throughput
    floor: DVE must touch every element twice (row pass + col pass);
    tensor_tensor_reduce (fused pass) hangs this HW, custom DVE ops
    and gpsimd alternatives are 1x or unavailable.
  * Tail: PE-transpose colacc 128x128 blocks into PSUM, 3D-view max
    reduces -> col mins; negate+relu (tensor_scalar), sqrt (ACT),
    row-sum; DMA [128,2] per core; host finishes the 128-way sums.
"""

import numpy as np
import ml_dtypes

B, N, M, P = 8, 4096, 4096, 128
NI = N // P        # 32 n-chunks
QW = 2048          # quad width (4 PSUM banks)
NQ = M // QW       # 2 quads per n-chunk
KROWS = 13         # augmented contraction rows
N_CORES = 8

_CACHE = {}


def _build_program():
    """Build + compile the SPMD bass program (same NEFF for all 8 cores)."""
    from contextlib import ExitStack
    import concourse.tile as tile
    from concourse import bacc, mybir

    f32 = mybir.dt.float32
    bf16 = mybir.dt.bfloat16
    mx = mybir.AluOpType.max
    X = mybir.AxisListType.X
    NEGBIG = -3.0e38

    nc = bacc.Bacc("TRN2", target_bir_lowering=False, debug=False,
                   num_devices=N_CORES)
    lhs_d = nc.dram_tensor("lhs", [P, N], bf16, kind="ExternalInput").ap()
    rhs_d = nc.dram_tensor("rhs", [P, M], bf16, kind="ExternalInput").ap()
    id_d = nc.dram_tensor("ident", [P, P], bf16, kind="ExternalInput").ap()
    out_d = nc.dram_tensor("out", [P, 2], f32, kind="ExternalOutput").ap()

    with tile.TileContext(nc) as tc:
        with ExitStack() as ctx:
            consts = ctx.enter_context(tc.tile_pool(name="consts", bufs=1))
            sqp = ctx.enter_context(tc.tile_pool(name="sq", bufs=2))
            treep = ctx.enter_context(tc.tile_pool(name="tree", bufs=2))

            lhs_sb = consts.tile([P, N], bf16)
            rhs_sb = consts.tile([P, M], bf16)
            for c in range(4):
                nc.sync.dma_start(lhs_sb[:, c * 1024:(c + 1) * 1024],
                                  lhs_d[:, c * 1024:(c + 1) * 1024])
                nc.sync.dma_start(rhs_sb[:, c * 1024:(c + 1) * 1024],
                                  rhs_d[:, c * 1024:(c + 1) * 1024])

            rowacc = consts.tile([P, NI], f32)      # per-chunk row max(-d2)
            colacc = consts.tile([P, M], bf16)      # running col max(-d2)
            nc.vector.memset(colacc, NEGBIG)
            ident = consts.tile([P, P], bf16)
            nc.sync.dma_start(ident, id_d)
            colmax_t = consts.tile([P, NI], f32)    # transposed col maxes
            sums = consts.tile([P, 2], f32)

            with tc.tile_pool(name="psumq", bufs=2, space="PSUM") as psq:
                for i in range(NI):
                    sq = sqp.tile([P, M], bf16, tag="sq")
                    for q in range(NQ):
                        quad = psq.tile([P, QW], f32, tag="quad")
                        for r in range(4):
                            mlo = q * QW + r * 512
                            nc.tensor.matmul(
                                quad[:, r * 512:(r + 1) * 512],
                                lhs_sb[32 * r:32 * r + KROWS,
                                       i * P:(i + 1) * P],
                                rhs_sb[32 * r:32 * r + KROWS,
                                       mlo:mlo + 512],
                                start=True, stop=True,
                                tile_position=(32 * r, 0),
                            )
                        nc.scalar.copy(sq[:, q * QW:(q + 1) * QW], quad)
                    # col direction: one elementwise max accumulate
                    nc.vector.tensor_tensor(colacc, colacc, sq, mx)
                    # row direction: bf16 max tree + 512-wide reduce
                    t1 = treep.tile([P, M // 2], bf16, tag="t1")
                    nc.vector.tensor_tensor(
                        t1, sq[:, :M // 2], sq[:, M // 2:], mx)
                    t2 = treep.tile([P, M // 4], bf16, tag="t2")
                    nc.vector.tensor_tensor(
                        t2, t1[:, :M // 4], t1[:, M // 4:], mx)
                    t3 = treep.tile([P, M // 8], bf16, tag="t3")
                    nc.vector.tensor_tensor(
                        t3, t2[:, :M // 8], t2[:, M // 8:], mx)
                    nc.vector.tensor_reduce(rowacc[:, i:i + 1], t3, X, mx)

            # tail: partition-max of colacc via PE transpose blocks,
            # 16 blocks per PSUM quad, one 3D-view reduce per quad.
            with tc.tile_pool(name="psumt", bufs=2, space="PSUM") as pst:
                for h in range(2):
                    tp = pst.tile([P, 16, P], bf16, tag="tp")
                    for t in range(16):
                        blk = h * 16 + t
                        nc.tensor.transpose(
                            tp[:, t, :],
                            colacc[:, blk * P:(blk + 1) * P], ident)
                    nc.vector.tensor_reduce(
                        colmax_t[:, h * 16:(h + 1) * 16], tp, X, mx)

                # negate+relu -> sqrt -> free-sum for both directions
                rrel = consts.tile([P, NI], f32)
                nc.vector.tensor_scalar(rrel, rowacc, -1.0, 0.0,
                                        mybir.AluOpType.mult, mx)
                rsqrt = consts.tile([P, NI], f32)
                nc.scalar.sqrt(rsqrt, rrel)
                nc.vector.tensor_reduce(
                    sums[:, 0:1], rsqrt, X, mybir.AluOpType.add)
                crel = consts.tile([P, NI], f32)
                nc.vector.tensor_scalar(crel, colmax_t, -1.0, 0.0,
                                        mybir.AluOpType.mult, mx)
                csqrt = consts.tile([P, NI], f32)
                nc.scalar.sqrt(csqrt, crel)
                nc.vector.tensor_reduce(
                    sums[:, 1:2], csqrt, X, mybir.AluOpType.add)

                # per-partition sums out; host finishes the 128-way sum
                nc.sync.dma_start(out_d, sums)

    nc.compile()
    return nc


def _host_prep(trgt, pred):
    """Per-batch augmented bf16 hi/lo matrices for NEGATED distances.

    -d2[n,m] = sum_k lhs[k,n]*rhs[k,m] with rows:
      k0-2 : 2 th_d    x  ph_d
      k3-5 : 2 th_d    x  pl_d
      k6-8 : 2 tl_d    x  ph_d
      k9,10: t2h, t2l  x  -1
      k11,12: 1        x  -p2h, -p2l
    """
    bf = ml_dtypes.bfloat16
    in_maps = []
    for b in range(B):
        t = np.asarray(trgt[b], dtype=np.float64)   # [N,3]
        p = np.asarray(pred[b], dtype=np.float64)   # [M,3]
        th = t.astype(bf).astype(np.float64)
        tl = (t - th).astype(bf).astype(np.float64)
        ph = p.astype(bf).astype(np.float64)
        pl = (p - ph).astype(bf).astype(np.float64)
        t2 = (t * t).sum(-1)
        p2 = (p * p).sum(-1)
        t2h = t2.astype(bf).astype(np.float64)
        t2l = (t2 - t2h).astype(bf).astype(np.float64)
        p2h = p2.astype(bf).astype(np.float64)
        p2l = (p2 - p2h).astype(bf).astype(np.float64)
        on = np.ones(N)
        lhs13 = np.stack([2 * th[:, 0], 2 * th[:, 1], 2 * th[:, 2],
                          2 * th[:, 0], 2 * th[:, 1], 2 * th[:, 2],
                          2 * tl[:, 0], 2 * tl[:, 1], 2 * tl[:, 2],
                          t2h, t2l, on, on])                    # [13,N]
        rhs13 = np.stack([ph[:, 0], ph[:, 1], ph[:, 2],
                          pl[:, 0], pl[:, 1], pl[:, 2],
                          ph[:, 0], ph[:, 1], ph[:, 2],
                          -on, -on, -p2h, -p2l])                # [13,M]
        lhs = np.zeros((P, N), dtype=bf)
        rhs = np.zeros((P, M), dtype=bf)
        for r in range(4):
            lhs[32 * r:32 * r + KROWS] = lhs13.astype(bf)
            rhs[32 * r:32 * r + KROWS] = rhs13.astype(bf)
        in_maps.append({"lhs": lhs, "rhs": rhs,
                        "ident": np.eye(P, dtype=np.float32).astype(bf)})
    return in_maps


def kernel(trgt, pred):
    from concourse.bass_utils import run_bass_kernel_spmd

    trgt = np.asarray(trgt, dtype=np.float32)
    pred = np.asarray(pred, dtype=np.float32)
    assert trgt.shape == (B, N, 3) and pred.shape == (B, M, 3)

    if "nc" not in _CACHE:
        _CACHE["nc"] = _build_program()
    nc = _CACHE["nc"]

    in_maps = _host_prep(trgt, pred)
    res = run_bass_kernel_spmd(nc, in_maps, list(range(N_CORES)))
    comp = np.zeros(B, dtype=np.float64)
    acc = np.zeros(B, dtype=np.float64)
    for b in range(B):
        o = np.asarray(res.results[b]["out"], dtype=np.float64)
        comp[b] = o[:, 0].sum() / N
        acc[b] = o[:, 1].sum() / N
    accuracy = np.float32(acc.mean())
    complete = np.float32(comp.mean())
    chamfer = np.float32(0.5 * (accuracy.astype(np.float64)
                                + complete.astype(np.float64)))
    return (accuracy, complete, chamfer)
